# revision 14
# baseline (speedup 1.0000x reference)
"""Trainium2 Bass kernel for nn_MultiHeadAttention_49246095016057.

Sharding: pure data parallelism — core b computes batch element b
(B=8 = n_cores, no collectives needed).

Per-core algorithm (S=1024, E=1024, H=16, Dh=64):
  - Host passes transposed operand layouts (x^T, W^T) so the device never
    transposes activations, plus the bool mask pre-converted to an additive
    -8e9*mask matrix in the [k,q] orientation.
  - Projections run on TensorE in float32r (TF32, 1 cyc/row at N=512):
      Q^T = Wk @ xq^T, K^T = Wv @ xk^T (written to DRAM, re-streamed per
      head), V = xv @ Wq^T kept natural [k,d] in bf16 with a ones column
      appended per head (softmax denominator trick).
  - Attention per head, scores kept transposed S^T[k,q]:
      S^T psum = K_h^T.T @ Q_h^T  (f32r)  + identity-matmul mask add (bf16)
      p~ = exp(0.125 * psum) on ScalarE -> bf16
      ctx~^T[65,q] = V1_h.T @ p~ accumulated over k tiles; row 64 = sum p~
      r = 1/row64; context^T rows = psum * r (-> f32r); averaged attention
      accumulates p~ * (r/16) in bf16 (split across VectorE / GpSimdE).
  - Out-projection from ctx^T (f32r) gives output in natural [q,e] layout;
    residual + mean fused via scalar_tensor_tensor accum_out; layernorm
    stats on per-partition scalars; apply via ScalarE scale/bias.
Outputs: y [q,e] fp32 and avg^T [k,q] fp32 (host transposes avg back).
"""

import os
import sys

import numpy as np

for _p in ("/opt/trn_rl_repo",):
    if _p not in sys.path:
        sys.path.append(_p)

import ml_dtypes
import concourse.bacc as bacc
import concourse.tile as tile
from concourse import mybir
from concourse.bass_utils import run_bass_kernel_spmd

F32 = mybir.dt.float32
F32R = mybir.dt.float32r
BF16 = mybir.dt.bfloat16

B, S, E = 8, 1024, 1024
H = 16
DH = E // H
NB = E // 128  # partition blocks per 1024 dim
EPS = 1e-5
MASK_NEG = -384.0  # becomes -48 after the 1/8 score scale inside exp (exp -> ~1e-21)

TRACE = bool(int(os.environ.get("KERNEL_TRACE", "0")))

_cache = {}


def _build(trivial_affine, zero_bias_qkv, zero_bias_o):
    PH = os.environ.get("KERNEL_PHASES", "123")
    nc = bacc.Bacc("TRN2", target_bir_lowering=False, debug=False, num_devices=8)

    xqT = nc.declare_dram_parameter("xqT", [E, S], F32R, isOutput=False)
    xkT = nc.declare_dram_parameter("xkT", [E, S], F32R, isOutput=False)
    xvT = nc.declare_dram_parameter("xvT", [E, S], F32R, isOutput=False)
    xq = nc.declare_dram_parameter("xq", [S, E], F32, isOutput=False)
    wkT = nc.declare_dram_parameter("wkT", [E, E], F32R, isOutput=False)
    wvT = nc.declare_dram_parameter("wvT", [E, E], F32R, isOutput=False)
    wqT = nc.declare_dram_parameter("wqT", [E, E], F32R, isOutput=False)
    woT = nc.declare_dram_parameter("woT", [E, E], BF16, isOutput=False)
    maskT = nc.declare_dram_parameter("maskT", [S, S], mybir.dt.float8e5, isOutput=False)
    ident = nc.declare_dram_parameter("ident", [128, 128], mybir.dt.float8e5, isOutput=False)
    bkp = nc.declare_dram_parameter("bkp", [E, 1], F32, isOutput=False)
    bvp = nc.declare_dram_parameter("bvp", [E, 1], F32, isOutput=False)
    bqr = nc.declare_dram_parameter("bqr", [1, E], F32, isOutput=False)
    bor = nc.declare_dram_parameter("bor", [1, E], F32, isOutput=False)
    gmr = nc.declare_dram_parameter("gmr", [1, E], F32, isOutput=False)
    btr = nc.declare_dram_parameter("btr", [1, E], F32, isOutput=False)

    y_out = nc.declare_dram_parameter("y", [S, E], F32, isOutput=True)
    avg_out = nc.declare_dram_parameter("avgT", [S, S], F32, isOutput=True)

    # internal DRAM round-trip tensors for Q^T / K^T
    qT_d = nc.dram_tensor("qT_d", [E, S], F32R)
    kT_d = nc.dram_tensor("kT_d", [E, S], F32R)

    def blk(dram2d, p=128):
        return dram2d.rearrange("(b p) s -> p b s", p=p)

    acts_t = {"q": blk(xqT), "k": blk(xkT), "v": blk(xvT)}
    w_t = {"q": blk(wkT), "k": blk(wvT), "v": blk(wqT), "o": blk(woT)}

    with tile.TileContext(nc) as tc:
        with (
            tc.tile_pool(name="big", bufs=1) as big,
            tc.tile_pool(name="x4", bufs=2) as x4,
            tc.tile_pool(name="qk", bufs=3) as qkp,
            tc.tile_pool(name="pp", bufs=8) as ppool,
            tc.tile_pool(name="st", bufs=1) as stp,
            tc.tile_pool(name="sm", bufs=2) as smp,
            tc.tile_pool(name="psA", bufs=2, space="PSUM") as psA,
            tc.tile_pool(name="psB", bufs=2, space="PSUM") as psB,
        ):
            # persistent tensors
            V1 = big.tile([128, NB, H, 66], BF16, tag="V1")
            MT = big.tile([128, NB, S], mybir.dt.float8e5, tag="MT")
            avgT = big.tile([128, NB, S], F32, tag="avgT")
            idt = big.tile([128, 128], mybir.dt.float8e5, tag="idt")
            nc.sync.dma_start(idt[:], ident[:])
            nc.gpsimd.memset(V1[:, :, :, 64:66], 1.0)
            for t in range(NB):
                nc.sync.dma_start(MT[:, t, :], blk(maskT)[:, t, :])

            if not zero_bias_qkv:
                bk_sb = big.tile([128, NB], F32, tag="bk")
                bv_sb = big.tile([128, NB], F32, tag="bv")
                nc.sync.dma_start(bk_sb[:], bkp.rearrange("(b p) o -> p (b o)", p=128))
                nc.sync.dma_start(bv_sb[:], bvp.rearrange("(b p) o -> p (b o)", p=128))
                bq_row = smp.tile([1, E], F32, tag="brow")
                nc.sync.dma_start(bq_row[:], bqr[:])
                bqb = big.tile([128, E], F32, tag="bqb")
                nc.gpsimd.partition_broadcast(bqb[:], bq_row[:])
            if not zero_bias_o:
                bo_row = smp.tile([1, E], F32, tag="brow")
                nc.sync.dma_start(bo_row[:], bor[:])
                bob = big.tile([128, E], F32, tag="bob")
                nc.gpsimd.partition_broadcast(bob[:], bo_row[:])
            if not trivial_affine:
                gm_row = smp.tile([1, E], F32, tag="brow")
                bt_row = smp.tile([1, E], F32, tag="brow")
                nc.sync.dma_start(gm_row[:], gmr[:])
                nc.sync.dma_start(bt_row[:], btr[:])
                gmb = big.tile([128, E], F32, tag="gmb")
                btb = big.tile([128, E], F32, tag="btb")
                nc.gpsimd.partition_broadcast(gmb[:], gm_row[:])
                nc.gpsimd.partition_broadcast(btb[:], bt_row[:])

            # ---------------- phase 1: projections ----------------
            for proj in ("q", "k", "v"):
                wt = x4.tile([128, NB, S], F32R, tag="x4")
                xt_ = x4.tile([128, NB, S], F32R, tag="x4")
                for t in range(NB):
                    nc.sync.dma_start(wt[:, t, :], w_t[proj][:, t, :])
                    nc.sync.dma_start(xt_[:, t, :], acts_t[proj][:, t, :])
                for m in range(NB):
                    pt = psA.tile([128, S], F32, tag="ps")
                    for kt in range(NB):
                        if proj == "v":
                            lhsT = xt_[:, kt, m * 128:(m + 1) * 128]
                            rhs = wt[:, kt, :]
                        else:
                            lhsT = wt[:, kt, m * 128:(m + 1) * 128]
                            rhs = xt_[:, kt, :]
                        for half in range(2):
                            hs = slice(half * 512, (half + 1) * 512)
                            nc.tensor.matmul(pt[:, hs], lhsT, rhs[:, hs],
                                             start=(kt == 0), stop=(kt == NB - 1))
                    if proj in ("q", "k"):
                        bounce = stp.tile([128, S], F32R, tag="s512e", bufs=2)
                        bias = 0.0 if zero_bias_qkv else (
                            bk_sb[:, m:m + 1] if proj == "q" else bv_sb[:, m:m + 1])
                        nc.scalar.activation(bounce[:], pt[:],
                                             mybir.ActivationFunctionType.Identity,
                                             bias=bias)
                        dst = blk(qT_d if proj == "q" else kT_d)
                        nc.sync.dma_start(dst[:, m, :], bounce[:])
                    else:
                        src = pt[:].rearrange("p (h d) -> p h d", h=H)
                        if zero_bias_qkv:
                            nc.vector.tensor_copy(V1[:, m, :, 0:64], src)
                        else:
                            nc.vector.tensor_add(
                                V1[:, m, :, 0:64], src,
                                bqb[:].rearrange("p (h d) -> p h d", h=H))

            # out-proj weights + context, loaded/filled during phase 2
            woTs = x4.tile([128, NB, S], BF16, tag="x4")
            for t in range(NB):
                nc.sync.dma_start(woTs[:, t, :], w_t["o"][:, t, :])
            ctxT = x4.tile([128, NB, S], BF16, tag="x4")

            # ---------------- phase 2: attention per head ----------------
            for h in (range(H) if "2" in PH else []):
                dblk = (DH * h) // 128
                p0 = (DH * h) % 128
                qh = qkp.tile([128, S], F32R, tag="qk")
                kh = qkp.tile([128, S], F32R, tag="qk")
                nc.sync.dma_start(qh[p0:p0 + DH, :], blk(qT_d, 64)[:, h, :])
                nc.sync.dma_start(kh[p0:p0 + DH, :], blk(kT_d, 64)[:, h, :])

                ctxp = psB.tile([65, S], F32, tag="ctx")
                ptiles = []
                for t in range(NB):
                    sp = psA.tile([128, S], F32, tag="ps")
                    lhsT = kh[p0:p0 + DH, t * 128:(t + 1) * 128]
                    rhs = qh[p0:p0 + DH, :]
                    for half in range(2):
                        hs = slice(half * 512, (half + 1) * 512)
                        nc.tensor.matmul(sp[:, hs], lhsT, rhs[:, hs],
                                         start=True, stop=False)
                        nc.tensor.matmul(sp[:, hs], idt[:], MT[:, t, hs],
                                         start=False, stop=True)
                    pt = ppool.tile([128, S], BF16, tag="p", bufs=8)
                    nc.scalar.activation(pt[:], sp[:],
                                         mybir.ActivationFunctionType.Exp,
                                         scale=0.125)
                    ptiles.append(pt)
                    for half in range(2):
                        hs = slice(half * 512, (half + 1) * 512)
                        nc.tensor.matmul(ctxp[:, hs], V1[:, t, h, 0:65], pt[:, hs],
                                         start=(t == 0), stop=(t == NB - 1))

                rrow = smp.tile([1, S], F32, tag="rrow", bufs=1)
                nc.vector.reciprocal(rrow[:], ctxp[64:65, :])
                rrow16 = smp.tile([1, S], BF16, tag="rrow16")
                nc.vector.tensor_scalar_mul(rrow16[:], rrow[:], 1.0 / H)
                rb = smp.tile([128, S], BF16, tag="rb")
                nc.gpsimd.partition_broadcast(rb[:], rrow16[:])
                rbf = smp.tile([64, S], F32, tag="rbf", bufs=1)
                nc.gpsimd.partition_broadcast(rbf[:], rrow[:])
                nc.vector.tensor_mul(ctxT[p0:p0 + DH, dblk, :], ctxp[0:64, :], rbf[:])

                for t in range(NB):
                    eng = nc.vector if t < 5 else nc.gpsimd
                    if h == 0:
                        eng.tensor_mul(avgT[:, t, :], ptiles[t][:], rb[:])
                    else:
                        ps_t = ppool.tile([128, S], F32, tag="pscale", bufs=2)
                        eng.tensor_mul(ps_t[:], ptiles[t][:], rb[:])
                        eng.tensor_add(avgT[:, t, :], avgT[:, t, :], ps_t[:])

            # ---------------- phase 3: out-proj + residual + layernorm ----------------
            for m in (range(NB) if "3" in PH else []):
                op = psA.tile([128, S], F32, tag="ps")
                for kt in range(NB):
                    lhsT = ctxT[:, kt, m * 128:(m + 1) * 128]
                    for half in range(2):
                        hs = slice(half * 512, (half + 1) * 512)
                        nc.tensor.matmul(op[:, hs], lhsT, woTs[:, kt, hs],
                                         start=(kt == 0), stop=(kt == NB - 1))
                _p3 = os.environ.get("KERNEL_P3", "full")
                if _p3.startswith("mm"):
                    yt0 = stp.tile([128, S], F32, tag="s512e", bufs=2)
                    if _p3 == "mmv":
                        nc.vector.tensor_copy(yt0[:], op[:])
                    else:
                        nc.scalar.copy(yt0[:], op[:])
                    nc.sync.dma_start(blk(y_out)[:, m, :], yt0[:])
                    if _p3 == "mm1" and m == 0:
                        break
                    continue
                xq_m = stp.tile([128, S], F32, tag="s512a", bufs=2)
                nc.sync.dma_start(xq_m[:], blk(xq)[:, m, :])
                xt = stp.tile([128, S], F32, tag="s512b", bufs=2)
                msum = smp.tile([128, 1], F32, tag="msum")
                if zero_bias_o:
                    nc.vector.scalar_tensor_tensor(
                        xt[:], op[:], 1.0, xq_m[:],
                        op0=mybir.AluOpType.mult, op1=mybir.AluOpType.add,
                        accum_out=msum[:])
                else:
                    x0 = stp.tile([128, S], F32, tag="s512c", bufs=1)
                    nc.vector.tensor_add(x0[:], op[:], xq_m[:])
                    nc.vector.scalar_tensor_tensor(
                        xt[:], x0[:], 1.0, bob[:],
                        op0=mybir.AluOpType.mult, op1=mybir.AluOpType.add,
                        accum_out=msum[:])
                if os.environ.get("KERNEL_P3", "full") == "resid":
                    nc.sync.dma_start(blk(y_out)[:, m, :], xt[:])
                    continue
                sqsum = smp.tile([128, 1], F32, tag="sqsum")
                xsq = stp.tile([128, S], F32, tag="s512c" if zero_bias_o else "s512d", bufs=1)
                nc.vector.scalar_tensor_tensor(
                    xsq[:], xt[:], 1.0, xt[:],
                    op0=mybir.AluOpType.mult, op1=mybir.AluOpType.mult,
                    accum_out=sqsum[:])
                if os.environ.get("KERNEL_P3", "full") == "var":
                    nc.sync.dma_start(blk(y_out)[:, m, :], xsq[:])
                    continue
                mu = smp.tile([128, 1], F32, tag="mu")
                nc.vector.tensor_scalar_mul(mu[:], msum[:], 1.0 / E)
                e2 = smp.tile([128, 1], F32, tag="e2")
                nc.vector.tensor_scalar_mul(e2[:], sqsum[:], 1.0 / E)
                mu2 = smp.tile([128, 1], F32, tag="mu2")
                nc.vector.tensor_mul(mu2[:], mu[:], mu[:])
                var = smp.tile([128, 1], F32, tag="var")
                nc.vector.tensor_sub(var[:], e2[:], mu2[:])
                vare = smp.tile([128, 1], F32, tag="vare")
                nc.vector.tensor_scalar_add(vare[:], var[:], EPS)
                sd = smp.tile([128, 1], F32, tag="sd")
                nc.scalar.activation(sd[:], vare[:],
                                     mybir.ActivationFunctionType.Sqrt,
                                     bias=0.0, scale=1.0)
                rstd = smp.tile([128, 1], F32, tag="rstd")
                nc.vector.reciprocal(rstd[:], sd[:])
                nmu = smp.tile([128, 1], F32, tag="nmu")
                nc.vector.scalar_tensor_tensor(
                    nmu[:], mu[:], -1.0, rstd[:],
                    op0=mybir.AluOpType.mult, op1=mybir.AluOpType.mult)
                yt = stp.tile([128, S], F32, tag="s512e", bufs=2)
                if os.environ.get("KERNEL_P3", "full") == "stats":
                    nc.vector.scalar_tensor_tensor(
                        yt[:], xt[:], rstd[:], xt[:],
                        op0=mybir.AluOpType.mult, op1=mybir.AluOpType.bypass)
                else:
                    nc.scalar.activation(yt[:], xt[:],
                                         mybir.ActivationFunctionType.Identity,
                                         bias=nmu[:], scale=rstd[:])
                if not trivial_affine:
                    y2 = stp.tile([128, S], F32, tag="s512f", bufs=2)
                    nc.vector.tensor_mul(y2[:], yt[:], gmb[:])
                    nc.vector.tensor_add(y2[:], y2[:], btb[:])
                    yt = y2
                nc.sync.dma_start(blk(y_out)[:, m, :], yt[:])

            # averaged attention write-out
            for t in (range(NB) if "2" in PH else []):
                nc.sync.dma_start(blk(avg_out)[:, t, :], avgT[:, t, :])

    nc.compile()
    return nc


def kernel(input_Q, input_K, input_V, attn_mask, Wk, bk, Wv, bv, Wq, bq, Wo, bo, gamma, beta):
    bf16 = ml_dtypes.bfloat16
    input_Q = np.asarray(input_Q, dtype=np.float32)
    input_K = np.asarray(input_K, dtype=np.float32)
    input_V = np.asarray(input_V, dtype=np.float32)
    attn_mask = np.asarray(attn_mask)
    Wk, bk = np.asarray(Wk, np.float32), np.asarray(bk, np.float32)
    Wv, bv = np.asarray(Wv, np.float32), np.asarray(bv, np.float32)
    Wq, bq = np.asarray(Wq, np.float32), np.asarray(bq, np.float32)
    Wo, bo = np.asarray(Wo, np.float32), np.asarray(bo, np.float32)
    gamma, beta = np.asarray(gamma, np.float32), np.asarray(beta, np.float32)

    trivial_affine = bool(np.all(gamma == 1.0) and np.all(beta == 0.0))
    zero_bias_qkv = bool(np.all(bk == 0) and np.all(bv == 0) and np.all(bq == 0))
    zero_bias_o = bool(np.all(bo == 0))

    key = (trivial_affine, zero_bias_qkv, zero_bias_o)
    if key not in _cache:
        _cache[key] = _build(*key)
    nc = _cache[key]

    wkT = np.ascontiguousarray(Wk.T)
    wvT = np.ascontiguousarray(Wv.T)
    wqT = np.ascontiguousarray(Wq.T)
    woT = np.ascontiguousarray(Wo.T).astype(ml_dtypes.bfloat16)
    identity = np.eye(128, dtype=np.float32).astype(ml_dtypes.float8_e5m2)
    shared = {
        "wkT": wkT, "wvT": wvT, "wqT": wqT, "woT": woT, "ident": identity,
        "bkp": np.ascontiguousarray(bk.reshape(E, 1)),
        "bvp": np.ascontiguousarray(bv.reshape(E, 1)),
        "bqr": np.ascontiguousarray(bq.reshape(1, E)),
        "bor": np.ascontiguousarray(bo.reshape(1, E)),
        "gmr": np.ascontiguousarray(gamma.reshape(1, E)),
        "btr": np.ascontiguousarray(beta.reshape(1, E)),
    }
    in_maps = []
    for b in range(B):
        m = dict(shared)
        m["xqT"] = np.ascontiguousarray(input_Q[b].T)
        m["xkT"] = np.ascontiguousarray(input_K[b].T)
        m["xvT"] = np.ascontiguousarray(input_V[b].T)
        m["xq"] = np.ascontiguousarray(input_Q[b])
        m["maskT"] = (attn_mask[b].T.astype(np.float32) * MASK_NEG).astype(ml_dtypes.float8_e5m2)
        in_maps.append(m)

    res = run_bass_kernel_spmd(nc, in_maps, core_ids=list(range(8)), trace=TRACE)
    if TRACE and res.exec_time_ns is not None:
        print(f"HW exec time: {res.exec_time_ns} ns", flush=True)
        kernel.last_exec_ns = res.exec_time_ns
        kernel.last_results = res

    y = np.stack([res.results[b]["y"] for b in range(B)])
    avg = np.stack([np.ascontiguousarray(res.results[b]["avgT"].T) for b in range(B)])
    return y, avg
